# revision 3
# baseline (speedup 1.0000x reference)
"""Causal depthwise conv1d (B=8, S=4096, H=2048, KS=4) on 8 trn2 NeuronCores.

Strategy (v2 — uint8 output wire):
  - Shard batch across the 8 cores (one batch element each, no halo needed).
  - bf16 x on the wire (host casts + transposes to (H, S)): 16 MiB in/core.
  - uint8 y on the wire: the kernel emits u8 = rne(y*r_c + 128) with a
    per-channel scale r_c = 126.5 / (sum_k |w_k,c| * max_s |x_c,s| + |b_c|)
    computed on the host and folded into every device-side weight. The host
    dequantizes y = (u8 - 128)/r_c. 8 MiB out/core instead of 16. Measured
    end-to-end max-rel-err ~1.0e-2 vs the 2e-2 gate (deterministic inputs).
  - Engine split per half-block (128 ch x 2048 cols):
      PE  : taps w0,w1,w2 as per-channel diagonal matmuls -> PSUM   ~2.8us
      ACT : e = ps + (b*r + 128)  (PSUM -> fp16 SBUF; fp16 so the
            +128-offset values keep a 0.125 quantum)                ~1.9us
      DVE : yq = rne(x*w3r + e) -> uint8, the fused tap3+merge+
            quantize scalar_tensor_tensor                           ~1.3-2.4us
      DMA : in 512KB + out 256KB                                    ~2.0us
    Every B_PERIOD-th half-block is a "B-half": PE drops tap0 and DVE (or
    GP) adds it via a second STT (x(-3)*w0r + e -> fp16), rebalancing PE.
  - Ring hygiene (from the bf16 baseline): x loads on the sync ring,
    stores + PAD memsets on gpsimd SWDGE, scalar ring = ACT only.
"""

import numpy as np

B, S, H, KS = 8, 4096, 2048, 4
NCORES = 8
PB = 128            # SBUF partitions
HB = H // PB        # 16 channel blocks per core
PAD = 4             # left zero-pad columns in the x tile (3 used + 1 align)
HW_ = 2048          # half-block width (PSUM tile = 4 banks)
BANK = 512          # PSUM bank width in f32 elements
NPE = 3             # taps held on PE (w0, w1, w2); w3 fused into the DVE STT

OFFSET = 128.0      # HW converts with round-to-nearest-even + saturate
B_PERIOD = 6        # every Nth half-block moves tap0 from PE to DVE/GP
GP_T03 = False      # True: B-half tap0 STT runs on gpsimd instead of DVE

RUN_KWARGS = {}
LAST_RESULTS = []

_cached = {}


def _build():
    import concourse.bacc as bacc
    import concourse.mybir as mybir
    import concourse.tile as tile

    f32 = mybir.dt.float32
    bf16 = mybir.dt.bfloat16
    fp16 = mybir.dt.float16
    u8 = mybir.dt.uint8
    Alu = mybir.AluOpType
    Act = mybir.ActivationFunctionType

    nc = bacc.Bacc(
        "TRN2",
        target_bir_lowering=False,
        debug=False,
        num_devices=NCORES,
    )
    xT = nc.dram_tensor("xT", [H, S], bf16, kind="ExternalInput")
    wp = nc.dram_tensor("wp", [PB, HB * 4], f32, kind="ExternalInput")
    wd = nc.dram_tensor("wd", [PB, HB * NPE * PB], bf16, kind="ExternalInput")
    yQ = nc.dram_tensor("yQ", [H, S], u8, kind="ExternalOutput")

    with tile.TileContext(nc) as tc:
        with tc.tile_pool(name="wpool", bufs=1) as wpool, \
             tc.tile_pool(name="xpool", bufs=5) as xpool, \
             tc.tile_pool(name="ypool", bufs=3) as ypool, \
             tc.tile_pool(name="data", bufs=4) as pool, \
             tc.tile_pool(name="ppool", bufs=2, space="PSUM") as ppool:
            # wp columns per hb: 0 = w3*r, 1 = b*r + OFFSET, 2 = w0*r, 3 = pad
            wsb = wpool.tile([PB, HB * 4], f32)
            wdb = wpool.tile([PB, HB * NPE * PB], bf16)
            nc.scalar.dma_start(wsb[:], wp[:])
            nc.scalar.dma_start(wdb[:], wd[:])
            # Tiny no-dep ACTIVATE so the ACT table load overlaps the first
            # x DMA instead of serializing in front of the first extraction.
            warm = wpool.tile([PB, 2], bf16)
            nc.vector.memset(warm[:], 0.0)
            nc.scalar.activation(warm[:], warm[:], Act.Identity, bias=0.0,
                                 scale=1.0)

            pend_store = []   # [(hb, y)] stores ~a block behind

            for hb in range(HB + 1):
                if hb < HB:
                    rows = slice(hb * PB, (hb + 1) * PB)
                    xt = xpool.tile([PB, PAD + S], bf16)
                    nc.gpsimd.memset(xt[:, 0:PAD], 0.0)
                    if hb == 0:
                        # first block in halves so compute starts ~1.5us in
                        nc.sync.dma_start(xt[:, PAD:PAD + HW_],
                                          xT[rows, 0:HW_])
                        nc.sync.dma_start(xt[:, PAD + HW_:PAD + S],
                                          xT[rows, HW_:S])
                    else:
                        nc.sync.dma_start(xt[:, PAD:PAD + S], xT[rows, :])
                    c = hb * 4
                    w3r = wsb[:, c + 0:c + 1]
                    bia = wsb[:, c + 1:c + 2]
                    w0r = wsb[:, c + 2:c + 3]
                    y = ypool.tile([PB, S], u8)
                    for half in range(S // HW_):
                        gidx = hb * 2 + half
                        s0 = half * HW_
                        base = PAD + s0
                        is_b = B_PERIOD and (gidx % B_PERIOD) == B_PERIOD - 1
                        ps = ppool.tile([PB, HW_], f32)
                        ks = list(range(1, NPE) if is_b else range(NPE))
                        for k in ks:
                            dcol = (hb * NPE + k) * PB
                            dw = wdb[:, dcol:dcol + PB]
                            shift = base - (NPE - k)  # k=0 -> s-3 .. k=2 -> s-1
                            for bk in range(HW_ // BANK):
                                nc.tensor.matmul(
                                    ps[:, bk * BANK:(bk + 1) * BANK],
                                    dw,
                                    xt[:, shift + bk * BANK:
                                           shift + (bk + 1) * BANK],
                                    start=(k == ks[0]), stop=(k == ks[-1]),
                                    skip_group_check=True)
                        # ACT: e = ps + bias'  (fp16 keeps 0.125 quantum)
                        e = pool.tile([PB, HW_], fp16, tag="e", bufs=4)
                        nc.scalar.activation(e[:], ps[:], Act.Identity,
                                             bias=bia, scale=1.0)
                        if is_b:
                            # tap0: t03 = x(-3)*w0r + e  (fp16)
                            t03 = pool.tile([PB, HW_], fp16, tag="t03",
                                            bufs=3)
                            eng = nc.gpsimd if GP_T03 else nc.vector
                            eng.scalar_tensor_tensor(
                                t03[:], xt[:, base - 3:base - 3 + HW_], w0r,
                                e[:], op0=Alu.mult, op1=Alu.add)
                            mrg = t03
                        else:
                            mrg = e
                        # fused tap3 + merge + quantize -> uint8
                        nc.vector.scalar_tensor_tensor(
                            y[:, s0:s0 + HW_], xt[:, base:base + HW_], w3r,
                            mrg[:], op0=Alu.mult, op1=Alu.add)
                        if half == 1 and pend_store:
                            phb, py = pend_store.pop(0)
                            prow = slice(phb * PB, (phb + 1) * PB)
                            nc.gpsimd.dma_start(yQ[prow, :], py[:])
                    pend_store.append((hb, y))
                else:
                    phb, py = pend_store.pop()
                    prow = slice(phb * PB, (phb + 1) * PB)
                    nc.gpsimd.dma_start(yQ[prow, :], py[:])
    nc.compile()
    return nc


def get_nc():
    if "nc" not in _cached:
        _cached["nc"] = _build()
    return _cached["nc"]


def core_scales(weight, bias, xT_bf):
    """Per-channel quant scale r (H,) for one core from its bf16 x (H,S)."""
    xmax = np.abs(xT_bf.astype(np.float32)).max(axis=1)          # (H,)
    bound = np.abs(weight).sum(axis=0) * xmax + np.abs(bias)
    return (126.5 / bound).astype(np.float32)


def pack_weights(weight, bias, r):
    wp = np.empty((PB, HB * 4), dtype=np.float32)
    w3r = weight[3] * r
    br = bias * r + OFFSET
    w0r = weight[0] * r
    for hb in range(HB):
        sl = slice(hb * PB, (hb + 1) * PB)
        wp[:, hb * 4 + 0] = w3r[sl]
        wp[:, hb * 4 + 1] = br[sl]
        wp[:, hb * 4 + 2] = w0r[sl]
        wp[:, hb * 4 + 3] = 0.0
    return wp


def pack_diag(weight, r):
    """Per-block diagonal matrices for taps w0..w2 (scaled by r), bf16."""
    import ml_dtypes
    wr = (weight[:NPE] * r[None, :]).astype(ml_dtypes.bfloat16)  # (NPE, H)
    wd = np.zeros((PB, HB * NPE * PB), dtype=ml_dtypes.bfloat16)
    idx = np.arange(PB)
    for hb in range(HB):
        for k in range(NPE):
            col = (hb * NPE + k) * PB
            wd[idx, col + idx] = wr[k, hb * PB + idx]
    return wd


def kernel(x, weight, bias):
    import ml_dtypes
    from concourse.bass_utils import run_bass_kernel_spmd

    x = np.asarray(x, dtype=np.float32)
    weight = np.asarray(weight, dtype=np.float32)
    bias = np.asarray(bias, dtype=np.float32)
    assert x.shape == (B, S, H), x.shape
    assert weight.shape == (KS, H), weight.shape
    assert bias.shape == (H,), bias.shape

    nc = get_nc()
    xT = x.transpose(0, 2, 1).astype(ml_dtypes.bfloat16)   # (B, H, S)
    rs, in_maps = [], []
    for i in range(NCORES):
        r = core_scales(weight, bias, xT[i])
        rs.append(r)
        in_maps.append({"xT": xT[i],
                        "wp": pack_weights(weight, bias, r),
                        "wd": pack_diag(weight, r)})
    try:
        res = run_bass_kernel_spmd(nc, in_maps, core_ids=list(range(NCORES)),
                                   **RUN_KWARGS)
    except Exception:
        res = run_bass_kernel_spmd(nc, in_maps, core_ids=list(range(NCORES)),
                                   **RUN_KWARGS)
    LAST_RESULTS.clear()
    LAST_RESULTS.append(res)
    out = np.empty((B, S, H), dtype=np.float32)
    for i in range(NCORES):
        u8v = res.results[i]["yQ"].astype(np.float32)      # (H, S)
        out[i] = ((u8v - 128.0) / rs[i][:, None]).T
    return out
